# revision 1
# baseline (speedup 1.0000x reference)
"""BiRWKV layer kernel for 8 Trainium2 NeuronCores.

Strategy (data-parallel over B=8, one batch element per core):
  - (channel, time) layout on chip: channels on the 128 SBUF partitions
    (C=512 -> 4 blocks), time on the free dim.
  - r/k/v projections for both directions are bf16 matmuls
    (lhsT = W block, rhs = x^T block) accumulated over 4 input-channel
    blocks into PSUM (fp32).
  - WKV runs UNSTABILIZED (mathematically equal to the reference's
    log-sum-exp form; values stay in range since |w|*T <= ~28, k~N(0,1)):
        den_t = d*den_{t-1} + e^{k_t};  num_t = d*num_{t-1} + e^{k_t} v_t
        y_t   = (num_{t-1} + e^{k_t+u} v_t) / (den_{t-1} + e^{k_t+u})
  - The den/num recurrences run on the DVE via tensor_tensor_scan over
    1024-wide time pairs (fp32 state, fp32 decay scalars, bf16 data);
    the backward direction uses reversed access patterns.
  - Division is exp(-ln(denom)) on the Scalar engine; ACT instructions
    are chained in program order and batched by table set (exp/tanh vs
    ln/exp) to avoid the 2.7us ACT table reload per switch.
  - sigmoid(r) = 0.5*(1+tanh(r/2)): 0.5 folded into W_out on the host,
    (1+tanh(r/2))*numer applied as one fused scalar_tensor_tensor.
  - Output projection consumes the (j2, t) activations directly as
    matmul lhsT; result (t, c) goes PSUM -> SBUF -> HBM.
"""

import numpy as np
import ml_dtypes

B, T, C = 8, 4096, 512
TT = 512           # time tile (psum width)
NTT = T // TT      # 8
CB = 4             # channel blocks
PW = 2 * TT        # pair width for SBUF-side elementwise
NP = T // PW       # 4 pairs
SBP = 2            # pairs per ACT-table sub-batch

_CACHE = {}


def _apply_tile_patches():
    """walrus in this container rejects instructions with >1 sync wait
    ("Too many sync wait commands"). Split excess waits onto same-engine
    nop carriers, and do the same for the TileContext tail drain."""
    import concourse.tile as tile_mod
    from concourse import mybir
    from concourse.vector_clock import ScopedClock

    if getattr(tile_mod, "_wait_split_patched", False):
        return
    MAXW = 1

    _orig_add = tile_mod.TileContext._add_instruction

    def _split_add(self, inst):
        si = inst.sync_info
        if si is not None and si.on_wait and len(si.on_wait) > MAXW:
            waits = list(si.on_wait)
            k = 0
            while len(waits) > MAXW:
                chunk, waits = waits[:MAXW], waits[MAXW:]
                carrier = mybir.InstNoOp(
                    name=f"{inst.name}_wsplit{k}",
                    engine=inst.engine,
                    bass_nofuse=True,
                    sync_info=mybir.SyncInfo(on_wait=chunk, on_update=[]),
                )
                k += 1
                _orig_add(self, carrier)
            inst.sync_info = mybir.SyncInfo(
                on_wait=waits, on_update=list(si.on_update)
            )
        return _orig_add(self, inst)

    def _drain_and_barrier(self, tick_clock, wait_clock):
        drain_inst = self.nc.sync.drain()
        wait_clock.add_sem_waits(
            drain_inst.ins, ScopedClock({None: tick_clock.global_clock})
        )
        si = drain_inst.ins.sync_info
        if si is not None and si.on_wait and len(si.on_wait) > MAXW:
            waits = list(si.on_wait)
            drain_inst.ins.sync_info = mybir.SyncInfo(
                on_wait=waits[:MAXW], on_update=list(si.on_update)
            )
            rest = waits[MAXW:]
            while rest:
                chunk, rest = rest[:MAXW], rest[MAXW:]
                n = self.nc.sync.nop(nofuse=True)
                n.ins.sync_info = mybir.SyncInfo(on_wait=chunk, on_update=[])

        self.nc.all_engine_barrier()
        assert self.sems is not None
        popped = self.nc._tile_sem_poison_stack.pop()
        assert popped is self._sem_poison
        self.nc.clear_and_free_semaphores(list(self.sems.allocated().values()))
        self.nc.all_engine_barrier()

    tile_mod.TileContext._add_instruction = _split_add
    tile_mod.TileContext._drain_and_barrier = _drain_and_barrier
    tile_mod._wait_split_patched = True


def _build_nc():
    import concourse.bass as bass
    import concourse.tile as tile
    from concourse import mybir
    from concourse.bass import _add_dep_helper

    _apply_tile_patches()

    f32 = mybir.dt.float32
    bf16 = mybir.dt.bfloat16
    Alu = mybir.AluOpType
    Act = mybir.ActivationFunctionType

    nc = bass.Bass()

    xT = nc.dram_tensor("xT", [C, T], bf16, kind="ExternalInput")
    wnames = ["w_rf", "w_kf", "w_vf", "w_rb", "w_kb", "w_vb"]
    wdram = {
        n: nc.dram_tensor(n, [128, 4 * C], bf16, kind="ExternalInput")
        for n in wnames
    }
    wout_d = nc.dram_tensor("wout", [128, 8 * C], bf16, kind="ExternalInput")
    u_f_d = nc.dram_tensor("u_f", [C, 1], f32, kind="ExternalInput")
    u_b_d = nc.dram_tensor("u_b", [C, 1], f32, kind="ExternalInput")
    eu_f_d = nc.dram_tensor("eu_f", [C, 1], f32, kind="ExternalInput")
    eu_b_d = nc.dram_tensor("eu_b", [C, 1], f32, kind="ExternalInput")
    dec_f_d = nc.dram_tensor("dec_f", [C, 1], f32, kind="ExternalInput")
    dec_b_d = nc.dram_tensor("dec_b", [C, 1], f32, kind="ExternalInput")
    out_d = nc.dram_tensor("y", [T, C], f32, kind="ExternalOutput")
    ypf_s = nc.dram_tensor("ypf_s", [C, T], bf16)  # fwd y_pre staging (HBM)

    # program-order chain for all ACT instructions (keeps the scheduler
    # from interleaving exp/tanh with ln, which would thrash the 2.7us
    # activation-table loads)
    act_state = {"last": None}

    def act(*args, **kwargs):
        i = nc.scalar.activation(*args, **kwargs)
        if act_state["last"] is not None:
            _add_dep_helper(i.ins, act_state["last"], False,
                            "ACT table-set program order")
        act_state["last"] = i.ins
        return i

    def act_copy(out, in_):
        i = nc.scalar.copy(out, in_)
        if act_state["last"] is not None:
            _add_dep_helper(i.ins, act_state["last"], False,
                            "ACT table-set program order")
        act_state["last"] = i.ins
        return i

    with tile.TileContext(nc) as tc:
        with (
            tc.tile_pool(name="wp", bufs=1) as wp,
            tc.tile_pool(name="cst", bufs=1) as cst,
            tc.tile_pool(name="chain", bufs=2) as chainp,
            tc.tile_pool(name="xt", bufs=2) as xtp,
            tc.tile_pool(name="wk", bufs=1) as wkp,
            tc.tile_pool(name="ps", bufs=1, space="PSUM") as psp,
        ):
            # ---- resident weights & constants ----
            wout = wp.tile([128, 8 * C], bf16, name="wout")
            nc.sync.dma_start(wout[:], wout_d[:])
            wt = {}
            for n in wnames:
                wt[n] = wp.tile([128, 4 * C], bf16, tag=n, name=n)
                nc.sync.dma_start(wt[n][:], wdram[n][:])
            u_t, eu_t, dec_t = {}, {}, {}
            for cb in range(CB):
                sl = slice(cb * 128, (cb + 1) * 128)
                for d, ud, eud, dd in (("f", u_f_d, eu_f_d, dec_f_d),
                                       ("b", u_b_d, eu_b_d, dec_b_d)):
                    u_t[(d, cb)] = cst.tile([128, 1], f32, tag=f"u{d}{cb}",
                                            name=f"u{d}{cb}")
                    nc.sync.dma_start(u_t[(d, cb)][:], ud[sl, :])
                    eu_t[(d, cb)] = cst.tile([128, 1], f32, tag=f"e{d}{cb}",
                                             name=f"e{d}{cb}")
                    nc.sync.dma_start(eu_t[(d, cb)][:], eud[sl, :])
                    dec_t[(d, cb)] = cst.tile([128, 1], f32, tag=f"d{d}{cb}",
                                              name=f"d{d}{cb}")
                    nc.sync.dma_start(dec_t[(d, cb)][:], dd[sl, :])

            def run_phase(d):
                fwd = d == "f"
                wr, wk, wv = wt["w_r" + d], wt["w_k" + d], wt["w_v" + d]
                pairs = list(range(NP)) if fwd else list(reversed(range(NP)))
                chains = {}

                def chain_buf(cb, kind):
                    key = (cb, kind)
                    t = chainp.tile([128, PW + 1], bf16,
                                    tag=f"ch_{kind}{cb}",
                                    name=f"ch_{kind}{cb}")
                    prev = chains.get(key)
                    chains[key] = t
                    if fwd:
                        if prev is None:
                            nc.vector.memset(t[:, 0:1], 0.0)
                        else:
                            nc.vector.tensor_copy(t[:, 0:1],
                                                  prev[:, PW: PW + 1])
                    else:
                        if prev is None:
                            nc.vector.memset(t[:, PW: PW + 1], 0.0)
                        else:
                            nc.vector.tensor_copy(t[:, PW: PW + 1],
                                                  prev[:, 0:1])
                    return t

                for pr in pairs:
                    p0 = pr * PW
                    # ---------------- part A ----------------
                    # x tiles for both halves up front (weights get reused
                    # across the two halves -> half the LDWEIGHTS)
                    xts = {}
                    for half, tt in enumerate((2 * pr, 2 * pr + 1)):
                        t0 = tt * TT
                        for kb in range(4):
                            xt = xtp.tile([128, TT], bf16, tag=f"xt{kb}h{half}",
                                          bufs=2, name=f"xt{kb}h{half}")
                            nc.sync.dma_start(
                                xt[:],
                                xT[kb * 128:(kb + 1) * 128, t0: t0 + TT])
                            xts[(half, kb)] = xt
                    stash = {}
                    for cb in range(CB):
                        ek, ekv, ekb, ekbv, th = {}, {}, {}, {}, {}
                        pss = {}
                        for cls, w in (("k", wk), ("v", wv), ("r", wr)):
                            for half in range(2):
                                pss[(cls, half)] = psp.tile(
                                    [128, TT], f32, tag=f"p{cls}", bufs=2,
                                    name=f"ps{cls}")
                            for kb in range(4):
                                wsl = w[:, kb * C + cb * 128:
                                        kb * C + cb * 128 + 128]
                                for half in range(2):
                                    nc.tensor.matmul(
                                        pss[(cls, half)][:], wsl,
                                        xts[(half, kb)][:],
                                        start=(kb == 0), stop=(kb == 3))
                        for half in range(2):
                            ek[half] = wkp.tile([128, TT], bf16, tag="ek",
                                                bufs=4, name="ek")
                            ekb[half] = wkp.tile([128, TT], bf16, tag="ekb",
                                                 bufs=4, name="ekb")
                            ekv[half] = wkp.tile([128, TT], bf16, tag="ekv",
                                                 bufs=4, name="ekv")
                            ekbv[half] = wkp.tile([128, TT], bf16, tag="ekbv",
                                                  bufs=4, name="ekbv")
                            th[half] = wkp.tile([128, TT], bf16, tag="th",
                                                bufs=4, name="th")
                            act(ek[half][:], pss[("k", half)][:], Act.Exp)
                            act(ekb[half][:], pss[("k", half)][:], Act.Exp,
                                bias=u_t[(d, cb)][:, 0:1])
                            act(th[half][:], pss[("r", half)][:], Act.Tanh,
                                bias=0.0, scale=0.5)
                            nc.vector.tensor_mul(ekv[half][:], ek[half][:],
                                                 pss[("v", half)][:])
                            nc.vector.tensor_mul(ekbv[half][:], ekb[half][:],
                                                 pss[("v", half)][:])
                        # scans at half width on standalone tiles
                        decbc = dec_t[(d, cb)][:, 0:1].broadcast_to([128, TT])
                        denb = chain_buf(cb, "den")
                        numb = chain_buf(cb, "num")
                        halves = (0, 1) if fwd else (1, 0)
                        for half in halves:
                            loc = half * TT
                            if fwd:
                                nc.vector.tensor_tensor_scan(
                                    denb[:, 1 + loc: 1 + loc + TT], decbc,
                                    ek[half][:], denb[:, loc: loc + 1],
                                    Alu.mult, Alu.add)
                                nc.vector.tensor_tensor_scan(
                                    numb[:, 1 + loc: 1 + loc + TT], decbc,
                                    ekv[half][:], numb[:, loc: loc + 1],
                                    Alu.mult, Alu.add)
                            else:
                                nc.vector.tensor_tensor_scan(
                                    denb[:, loc: loc + TT][:, ::-1], decbc,
                                    ek[half][:][:, ::-1],
                                    denb[:, loc + TT: loc + TT + 1],
                                    Alu.mult, Alu.add)
                                nc.vector.tensor_tensor_scan(
                                    numb[:, loc: loc + TT][:, ::-1], decbc,
                                    ekv[half][:][:, ::-1],
                                    numb[:, loc + TT: loc + TT + 1],
                                    Alu.mult, Alu.add)
                        if fwd:
                            den_prev = denb[:, 0:PW]
                            num_prev = numb[:, 0:PW]
                        else:
                            den_prev = denb[:, 1: 1 + PW]
                            num_prev = numb[:, 1: 1 + PW]
                        dnm = wkp.tile([128, PW], bf16, tag="dnm",
                                       bufs=6, name="dnm")
                        nmr = wkp.tile([128, PW], bf16, tag="nmr",
                                       bufs=3, name="nmr")
                        nmr2 = wkp.tile([128, PW], bf16, tag="nmr2",
                                        bufs=6, name="nmr2")
                        for half in range(2):
                            hs = slice(half * TT, (half + 1) * TT)
                            nc.gpsimd.tensor_add(dnm[:, hs], ekb[half][:],
                                                 den_prev[:, hs])
                            nc.gpsimd.tensor_add(nmr[:, hs], ekbv[half][:],
                                                 num_prev[:, hs])
                            nc.vector.scalar_tensor_tensor(
                                nmr2[:, hs], th[half][:], 1.0, nmr[:, hs],
                                Alu.add, Alu.mult)
                        stash[cb] = (dnm, nmr2)

                    # ---------------- part B ----------------
                    ypb_tiles = {}
                    for cb in range(CB):
                        dnm, nmr2 = stash[cb]
                        lnb = wkp.tile([128, PW], f32, tag="lnb", bufs=3,
                                       name="lnb")
                        act(lnb[:], dnm[:], Act.Ln)
                        invb = wkp.tile([128, PW], bf16, tag="invb",
                                        bufs=2, name="invb")
                        act(invb[:], lnb[:], Act.Exp, scale=-1.0)
                        yb = wkp.tile([128, PW], bf16, tag="ypb",
                                      bufs=6, name="ypb")
                        nc.gpsimd.tensor_mul(yb[:], nmr2[:], invb[:])
                        if fwd:
                            nc.sync.dma_start(
                                ypf_s[cb * 128:(cb + 1) * 128, p0: p0 + PW],
                                yb[:])
                        else:
                            ypb_tiles[cb] = yb

                    # ---------------- part C (bwd only) ----------------
                    if not fwd:
                        ypfl = {}
                        for cb in range(CB):
                            ypfl[cb] = wkp.tile([128, PW], bf16,
                                                tag=f"ypfl{cb}", bufs=2,
                                                name=f"ypfl{cb}")
                            nc.sync.dma_start(
                                ypfl[cb][:],
                                ypf_s[cb * 128:(cb + 1) * 128, p0: p0 + PW])
                        for m in range(PW // 128):
                            t0 = p0 + m * 128
                            pso = psp.tile([128, C], f32, tag="po",
                                           bufs=2, name="pso")
                            for cb in range(CB):
                                nc.tensor.matmul(
                                    pso[:],
                                    ypfl[cb][:, m * 128: (m + 1) * 128],
                                    wout[:, cb * C: (cb + 1) * C],
                                    start=(cb == 0), stop=False)
                            for cb in range(CB):
                                nc.tensor.matmul(
                                    pso[:],
                                    ypb_tiles[cb][:, m * 128: (m + 1) * 128],
                                    wout[:, (4 + cb) * C: (5 + cb) * C],
                                    start=False, stop=(cb == 3))
                            osb = wkp.tile([128, C], f32, tag="osb",
                                           bufs=2, name="osb")
                            act_copy(osb[:], pso[:])
                            nc.sync.dma_start(
                                out_d[t0: t0 + 128, :], osb[:])

            run_phase("f")
            run_phase("b")

    return nc


def _host_prep(x, W_rkv, W_out, time_decay, time_first, time_decay_rev,
               time_first_rev):
    bf16 = ml_dtypes.bfloat16
    f32 = np.float32

    Wr = W_rkv.reshape(C, 2, 3, C)
    pieces = {
        "w_rf": Wr[:, 0, 0], "w_kf": Wr[:, 0, 1], "w_vf": Wr[:, 0, 2],
        "w_rb": Wr[:, 1, 0], "w_kb": Wr[:, 1, 1], "w_vb": Wr[:, 1, 2],
    }
    wmaps = {}
    for n, p in pieces.items():
        wmaps[n] = np.ascontiguousarray(
            p.reshape(4, 128, C).transpose(1, 0, 2).reshape(128, 4 * C)
        ).astype(bf16)

    Wo = (0.5 * W_out).reshape(8, 128, C).transpose(1, 0, 2).reshape(128, 8 * C)
    wout = np.ascontiguousarray(Wo).astype(bf16)

    u_f = np.ascontiguousarray(time_first.reshape(C, 1)).astype(f32)
    u_b = np.ascontiguousarray(time_first_rev.reshape(C, 1)).astype(f32)
    eu_f = np.exp(time_first.astype(np.float64)).reshape(C, 1).astype(f32)
    eu_b = np.exp(time_first_rev.astype(np.float64)).reshape(C, 1).astype(f32)
    dec_f = np.exp(-np.exp(time_decay.astype(np.float64))).reshape(C, 1).astype(f32)
    dec_b = np.exp(-np.exp(time_decay_rev.astype(np.float64))).reshape(C, 1).astype(f32)

    shared = dict(wout=wout, u_f=u_f, u_b=u_b, eu_f=eu_f, eu_b=eu_b,
                  dec_f=dec_f, dec_b=dec_b, **wmaps)
    in_maps = []
    for b in range(B):
        m = dict(shared)
        m["xT"] = np.ascontiguousarray(x[b].T).astype(bf16)
        in_maps.append(m)
    return in_maps


def kernel(x, W_rkv, W_out, time_decay, time_first, time_decay_rev,
           time_first_rev, _trace=False):
    from concourse.bass_utils import run_bass_kernel_spmd

    x = np.asarray(x, dtype=np.float32)
    W_rkv = np.asarray(W_rkv, dtype=np.float32)
    W_out = np.asarray(W_out, dtype=np.float32)
    time_decay = np.asarray(time_decay, dtype=np.float32)
    time_first = np.asarray(time_first, dtype=np.float32)
    time_decay_rev = np.asarray(time_decay_rev, dtype=np.float32)
    time_first_rev = np.asarray(time_first_rev, dtype=np.float32)

    if "nc" not in _CACHE:
        _CACHE["nc"] = _build_nc()
    nc = _CACHE["nc"]

    in_maps = _host_prep(x, W_rkv, W_out, time_decay, time_first,
                         time_decay_rev, time_first_rev)
    res = run_bass_kernel_spmd(
        nc, in_maps, core_ids=list(range(B)), trace=_trace
    )
    _CACHE["last_result"] = res
    out = np.stack([res.results[b]["y"].astype(np.float32) for b in range(B)])
    return out

